# revision 19
# baseline (speedup 1.0000x reference)
"""GNN message-passing (std aggregator) on 8 TRN2 NeuronCores — pair-gather.

Like kernel.py round 3, but each dma_gather descriptor fetches a 512B element
holding TWO adjacent nodes' [x | x^2] bf16 rows (idx = src>>1), halving the Q7
descriptor-generation work (the kernel's bottleneck). The host-shipped one-hot
tiles are [128, 256] per column: slot s for even-src edges, 128+s for odd-src
edges; each column is reduced with two matmuls (even half, odd half) into the
same PSUM accumulator.

Blocks split edges into NRUN=2 equal-count src-sorted runs (pair-space windows
of 32768 pairs cover the int16 idx range); each (group, run) gather stream is
split in half across two SWDGE queues so all 4 Q7 core-pairs stay busy.
"""

import numpy as np

N_NODES = 100000
N_FEAT = 64
N_EDGES = 1600000
P = 128
NCORES = 8
NB = 98                 # blocks per core
NBLK = NCORES * NB      # 784
GB = 7                  # blocks per group; 98 = 14*7
NRUN = 2                # equal-count src-sorted runs per block
NPAIR = N_NODES // 2
WINP = 32768            # gather window rows in pair units (int16 idx space)
MM_DT = "bfloat16"
OH_DT = "float8e4"
FP8_ONE = 0x38          # float8_e4m3 bit pattern of 1.0

_CACHE = {}


def _build_program(n_nodes, f, nb, tq, gb, bases, mm_dt, oh_dt):
    import concourse.bacc as bacc
    import concourse.mybir as mybir
    import concourse.tile as tile

    F32 = mybir.dt.float32
    I16 = mybir.dt.int16
    MDT = getattr(mybir.dt, mm_dt)
    ODT = getattr(mybir.dt, oh_dt)
    AO = mybir.AluOpType

    t = NRUN * tq              # tiles (columns) per block
    W = 2 * f                  # 128
    E2 = 2 * W                 # 256: pair element width
    C = nb * t                 # total columns per core
    gcols = gb * t             # columns per group
    rcols = gb * tq            # columns per (group, run)
    hcols = rcols // 2         # columns per half-gather
    ng = nb // gb
    nidx = hcols * P           # indices per (half) gather
    i16c = nidx // 16          # idx16 cols per gather

    assert rcols % 2 == 0

    nc = bacc.Bacc(num_swdge_queues=4)
    xxd = nc.declare_dram_parameter("xx", [NPAIR, E2], MDT, isOutput=False)
    gidxd = nc.declare_dram_parameter(
        "gidx", [P, ng * NRUN * 2 * i16c], I16, isOutput=False)
    ohd = nc.declare_dram_parameter("oh", [P, C * E2], ODT, isOutput=False)
    recd = nc.declare_dram_parameter("rec", [P, nb], F32, isOutput=False)
    outd = nc.declare_dram_parameter("out", [nb * P, f], F32, isOutput=True)

    with tile.TileContext(nc) as tc:
        with (
            tc.tile_pool(name="const", bufs=1) as constp,
            tc.tile_pool(name="io", bufs=2) as iop,
            tc.tile_pool(name="msg", bufs=2) as msgp,
            tc.tile_pool(name="fin", bufs=2) as finp,
            tc.tile_pool(name="ov", bufs=2) as ovp,
            tc.tile_pool(name="ps", bufs=8, space="PSUM") as psump,
        ):
            recb = constp.tile([P, nb], F32)
            nc.sync.dma_start(out=recb[:], in_=recd[:, :])

            for g in range(ng):
                idx = iop.tile([P, NRUN * 2 * i16c], I16, tag="idx")
                nc.sync.dma_start(
                    out=idx[:],
                    in_=gidxd[:, g * NRUN * 2 * i16c:
                              (g + 1) * NRUN * 2 * i16c])
                sqx = msgp.tile([P, gcols * E2], MDT, tag="sqx")
                s3 = sqx[:].rearrange("p (c w) -> p c w", w=E2)
                for k in range(NRUN):
                    for h in range(2):
                        q = k * 2 + h
                        c0 = k * rcols + h * hcols
                        nc.gpsimd.dma_gather(
                            out_ap=s3[:, c0:c0 + hcols, :],
                            in_ap=xxd[bases[k]:bases[k] + WINP, :],
                            idxs_ap=idx[:, q * i16c:(q + 1) * i16c],
                            num_idxs=nidx,
                            num_idxs_reg=nidx,
                            elem_size=E2,
                            single_packet=False,
                            queue_num=q,
                        )
                ohg = msgp.tile([P, gcols * E2], ODT, tag="ohg")
                nc.sync.dma_start(
                    out=ohg[:], in_=ohd[:, g * gcols * E2:(g + 1) * gcols * E2])

                pss = [psump.tile([P, W], F32, tag="ps", name=f"ps_{g}_{bl}")
                       for bl in range(gb)]
                for cl in range(gcols):
                    k = cl // rcols
                    r = cl % rcols
                    bl = r // tq
                    j = r % tq
                    nc.tensor.matmul(
                        out=pss[bl][:],
                        lhsT=ohg[:, cl * E2:cl * E2 + P],
                        rhs=sqx[:, cl * E2:cl * E2 + W],
                        start=(k == 0 and j == 0),
                        stop=False,
                    )
                    nc.tensor.matmul(
                        out=pss[bl][:],
                        lhsT=ohg[:, cl * E2 + P:(cl + 1) * E2],
                        rhs=sqx[:, cl * E2 + W:(cl + 1) * E2],
                        start=False,
                        stop=(k == NRUN - 1 and j == tq - 1),
                    )
                # finishing for the whole group: [S1|S2] -> std
                me = finp.tile([P, gb * W], F32, tag="me")
                for bl in range(gb):
                    nc.vector.tensor_scalar_mul(
                        out=me[:, bl * W:(bl + 1) * W], in0=pss[bl][:],
                        scalar1=recb[:, g * gb + bl:g * gb + bl + 1])
                m3 = me[:].rearrange("p (c w) -> p c w", w=W)
                sqm = finp.tile([P, gb * f], F32, tag="sqm")
                q3 = sqm[:].rearrange("p (c e) -> p c e", e=f)
                nc.scalar.square(out=q3[:, :, :], in_=m3[:, :, 0:f])
                va = finp.tile([P, gb * f], F32, tag="va")
                v3 = va[:].rearrange("p (c e) -> p c e", e=f)
                nc.vector.tensor_tensor(
                    out=v3[:, :, :], in0=m3[:, :, f:W], in1=q3[:, :, :],
                    op=AO.subtract)
                nc.vector.tensor_scalar(
                    out=va[:], in0=va[:], scalar1=0.0, scalar2=None,
                    op0=AO.max)
                so = ovp.tile([P, gb * f], F32, tag="so")
                nc.scalar.sqrt(out=so[:], in_=va[:])
                nc.sync.dma_start(
                    out=outd[g * gb * P:(g + 1) * gb * P, :]
                        .rearrange("(c p) e -> p c e", p=P),
                    in_=so[:].rearrange("p (c e) -> p c e", e=f))
    return nc


def _host_prep(x, edge_index):
    import ml_dtypes

    src = np.asarray(edge_index[0], dtype=np.int64)
    tgt = np.asarray(edge_index[1], dtype=np.int64)
    n_edges = src.shape[0]
    counts = np.bincount(tgt, minlength=N_NODES)

    # serpentine deal of count-sorted nodes into NBLK blocks of <=128 slots
    order = np.argsort(-counts, kind="stable")
    ranks = np.arange(N_NODES)
    rounds = ranks // NBLK
    pos = ranks % NBLK
    blk_of_rank = np.where(rounds % 2 == 0, pos, NBLK - 1 - pos)
    blk = np.empty(N_NODES, np.int64)
    slot = np.empty(N_NODES, np.int64)
    blk[order] = blk_of_rank
    slot[order] = rounds
    assert slot.max() < P

    eb = blk[tgt]
    es = slot[tgt]
    bc = np.bincount(eb, minlength=NBLK)
    starts_b = np.zeros(NBLK, np.int64)
    np.cumsum(bc[:-1], out=starts_b[1:])

    # per block: sort edges by src, split into NRUN equal-count runs
    order_e = np.lexsort((src, eb))
    sb = src[order_e]
    ebo = eb[order_e]
    eso = es[order_e]
    r = np.arange(n_edges) - starts_b[ebo]
    nb_of_e = bc[ebo]
    k = (r * NRUN) // nb_of_e
    w = r - (k * nb_of_e + NRUN - 1) // NRUN
    run_sizes = np.bincount(ebo * NRUN + k, minlength=NBLK * NRUN)
    tq = int(np.ceil(run_sizes.max() / P))
    cap = tq * P
    assert (w >= 0).all() and (w < cap).all()

    # static gather-window bases per run, in pair units
    pb = sb >> 1
    bases = []
    for kk in range(NRUN):
        m = k == kk
        lo = int(pb[m].min())
        hi = int(pb[m].max())
        base = min(lo, NPAIR - WINP)
        assert base >= 0 and hi - base < WINP, (kk, lo, hi, base)
        bases.append(base)
    bases = tuple(bases)

    flat = (ebo * NRUN + k) * cap + w
    gidxq = np.zeros((NBLK, NRUN, cap), np.int16)
    gidxq.reshape(-1)[flat] = (pb - np.asarray(bases)[k]).astype(np.int16)

    # rec table: [slot, block] -> 1/count (0 where count<=1 or empty slot)
    rec = np.where(counts >= 2, 1.0 / np.maximum(counts, 1), 0.0)
    recq = np.zeros((NBLK, P), np.float32)
    recq[blk, slot] = rec.astype(np.float32)

    # paired rhs table [x_even | x^2_even | x_odd | x^2_odd] in bf16
    xf = np.asarray(x, dtype=np.float32)
    W = 2 * N_FEAT
    xx = np.empty((NPAIR, 2 * W), ml_dtypes.bfloat16)
    xx[:, 0:N_FEAT] = xf[0::2]
    xx[:, N_FEAT:W] = xf[0::2] * xf[0::2]
    xx[:, W:W + N_FEAT] = xf[1::2]
    xx[:, W + N_FEAT:] = xf[1::2] * xf[1::2]
    xx = np.ascontiguousarray(xx)

    ng = NB // GB
    hcap = (GB * cap) // 2
    i16c = hcap // 16
    C = NB * NRUN * tq
    E2 = 2 * W

    # one-hot tiles [P, C, 256] fp8: slot + 128*parity(src)
    core_e = ebo // NB
    bloc = ebo % NB
    g_e = bloc // GB
    bl_e = bloc % GB
    j_e = w // P
    p_e = w % P
    cl_e = g_e * (NRUN * GB * tq) + k * (GB * tq) + bl_e * tq + j_e
    flat_oh = p_e * (C * E2) + cl_e * E2 + eso + P * (sb & 1)

    in_maps = []
    for c in range(NCORES):
        gi = gidxq[c * NB:(c + 1) * NB]
        # streams per (group, run, half): GB*cap split into 2 halves
        gs = (gi.reshape(ng, GB, NRUN, cap)
              .transpose(0, 2, 1, 3)                 # [ng, NRUN, GB, cap]
              .reshape(ng * NRUN * 2, hcap))         # per-half-gather streams
        idx16 = np.ascontiguousarray(
            np.tile(gs.reshape(ng * NRUN * 2, i16c, 16).transpose(0, 2, 1)
                    .reshape(ng * NRUN * 2 * 16, i16c)
                    .reshape(ng * NRUN * 2, 16, i16c)
                    .transpose(1, 0, 2).reshape(16, ng * NRUN * 2 * i16c),
                    (8, 1)))
        oh_u8 = np.zeros(P * C * E2, np.uint8)
        oh_u8[flat_oh[core_e == c]] = FP8_ONE
        oh = oh_u8.view(ml_dtypes.float8_e4m3).reshape(P, C * E2)
        in_maps.append({
            "xx": xx,
            "gidx": idx16,
            "oh": oh,
            "rec": np.ascontiguousarray(recq[c * NB:(c + 1) * NB].T),
        })
    return tq, bases, in_maps, blk, slot


def _run(x, edge_index, trace=False):
    from concourse.bass_utils import run_bass_kernel_spmd

    tq, bases, in_maps, blk, slot = _host_prep(x, edge_index)
    key = ("prog", tq, bases, MM_DT, OH_DT)
    if key not in _CACHE:
        nc_ = _build_program(N_NODES, N_FEAT, NB, tq, GB, bases, MM_DT, OH_DT)
        nc_.finalize()
        _CACHE[key] = nc_
    nc = _CACHE[key]
    res = run_bass_kernel_spmd(
        nc, in_maps, core_ids=list(range(NCORES)), trace=trace)

    outs = [np.asarray(r["out"]) for r in res.results]
    out_full = np.empty((N_NODES, N_FEAT), np.float32)
    cores = blk // NB
    rows = (blk % NB) * P + slot
    for c in range(NCORES):
        m = cores == c
        out_full[m] = outs[c][rows[m]]
    return out_full, res


def kernel(**inputs):
    out, _ = _run(inputs["x"], inputs["edge_index"], trace=False)
    return out


# revision 21
# speedup vs baseline: 1.0920x; 1.0920x over previous
"""GNN message-passing (std aggregator) on 8 TRN2 NeuronCores.

Math per target node: count, S1 = sum x[src], S2 = sum x[src]^2;
mean = S1/count; var = S2/count - mean^2; std = sqrt(max(var,0)),
zeroed where count <= 1 (host folds the mask into rec = 1/count).

Strategy: shard TARGET nodes across cores (no collectives). Host packs nodes
into 128-bin blocks balanced by in-degree (serpentine deal). Per block, edges
are sorted by src and split into NRUN equal-count runs; run k of all blocks is
gathered from a 32768-row window (base_k from the data) so int16 gather
indices suffice with ~0 capacity padding. Host also ships (a) an interleaved
bf16 xx = [x | x^2] table so one dma_gather descriptor (256B) fetches a
ready-made rhs row, and (b) the per-edge one-hot routing tiles pre-encoded in
fp8e4 so no engine has to build them. Device per core, per group of GB blocks:
  - 4x dma_gather (one per run; one SWDGE queue each so the 4 Q7 core-pairs
    generate descriptors in parallel) pulls xx[src] rows into SBUF as the
    matmul rhs,
  - a sequential DMA loads the fp8 one-hot tiles,
  - PE matmul-accumulates [128 slots x 128] = [S1 | S2] in per-block PSUM
    banks (zero-region = 2KB bank, so one open group per bank),
  - finishing: per-block PSUM->SBUF copy fused with the 1/count multiply,
    then batched var/sqrt and one DMA out per group.
"""

import numpy as np

N_NODES = 100000
N_FEAT = 64
N_EDGES = 1600000
P = 128
NCORES = 8
NB = 98                 # blocks per core
NBLK = NCORES * NB      # 784
GB = 7                  # blocks per group; 98 = 14*7
NRUN = 4                # equal-count src-sorted runs per block
WIN = 32768             # gather window rows (int16 idx space)
MM_DT = "bfloat16"      # rhs dtype for matmul
OH_DT = "float8e4"      # one-hot dtype (0/1 exact)
FP8_ONE = 0x38          # float8_e4m3 bit pattern of 1.0

_CACHE = {}


def _build_program(n_nodes, f, nb, tq, gb, bases, mm_dt, oh_dt):
    import concourse.bacc as bacc
    import concourse.mybir as mybir
    import concourse.tile as tile

    F32 = mybir.dt.float32
    I16 = mybir.dt.int16
    MDT = getattr(mybir.dt, mm_dt)
    ODT = getattr(mybir.dt, oh_dt)
    AO = mybir.AluOpType

    t = NRUN * tq              # tiles (columns) per block
    W = 2 * f                  # 128
    C = nb * t                 # total columns per core
    gcols = gb * t             # columns per group
    rcols = gb * tq            # columns per (group, run)
    ng = nb // gb
    nidx = rcols * P           # indices per gather
    i16c = nidx // 16          # idx16 cols per gather

    nc = bacc.Bacc(num_swdge_queues=4)
    xxd = nc.declare_dram_parameter("xx", [n_nodes, W], MDT, isOutput=False)
    gidxd = nc.declare_dram_parameter(
        "gidx", [P, ng * NRUN * i16c], I16, isOutput=False)
    ohd = nc.declare_dram_parameter("oh", [P, C * P], ODT, isOutput=False)
    recd = nc.declare_dram_parameter("rec", [P, nb], F32, isOutput=False)
    outd = nc.declare_dram_parameter("out", [nb * P, f], F32, isOutput=True)

    with tile.TileContext(nc) as tc:
        with (
            tc.tile_pool(name="const", bufs=1) as constp,
            tc.tile_pool(name="io", bufs=2) as iop,
            tc.tile_pool(name="msg", bufs=2) as msgp,
            tc.tile_pool(name="fin", bufs=2) as finp,
            tc.tile_pool(name="ov", bufs=2) as ovp,
            tc.tile_pool(name="ps", bufs=8, space="PSUM") as psump,
        ):
            # prefetch the int16 index table before anything else so the
            # first gathers start as early as possible; group 0's slice is
            # its own small DMA so it lands first
            idxall = constp.tile([P, ng * NRUN * i16c], I16)
            nc.sync.dma_start(out=idxall[:, :NRUN * i16c],
                              in_=gidxd[:, :NRUN * i16c])
            nc.sync.dma_start(out=idxall[:, NRUN * i16c:],
                              in_=gidxd[:, NRUN * i16c:])
            recb = constp.tile([P, nb], F32)
            nc.sync.dma_start(out=recb[:], in_=recd[:, :])

            for g in range(ng):
                sqx = msgp.tile([P, gcols * W], MDT, tag="sqx")
                s3 = sqx[:].rearrange("p (c w) -> p c w", w=W)
                for k in range(NRUN):
                    nc.gpsimd.dma_gather(
                        out_ap=s3[:, k * rcols:(k + 1) * rcols, :],
                        in_ap=xxd[bases[k]:bases[k] + WIN, :],
                        idxs_ap=idxall[:, (g * NRUN + k) * i16c:
                                       (g * NRUN + k + 1) * i16c],
                        num_idxs=nidx,
                        num_idxs_reg=nidx,
                        elem_size=W,
                        single_packet=False,
                        queue_num=k,
                    )
                ohg = msgp.tile([P, gcols * P], ODT, tag="ohg")
                nc.sync.dma_start(
                    out=ohg[:], in_=ohd[:, g * gcols * P:(g + 1) * gcols * P])
                pss = [psump.tile([P, W], F32, tag="ps", name=f"ps_{g}_{bl}")
                       for bl in range(gb)]
                for cl in range(gcols):
                    k = cl // rcols
                    r = cl % rcols
                    bl = r // tq
                    j = r % tq
                    nc.tensor.matmul(
                        out=pss[bl][:],
                        lhsT=ohg[:, cl * P:(cl + 1) * P],
                        rhs=sqx[:, cl * W:(cl + 1) * W],
                        start=(k == 0 and j == 0),
                        stop=(k == NRUN - 1 and j == tq - 1),
                    )
                # finishing for the whole group: [S1|S2] -> std
                # psum -> sbuf copy fused with the 1/count multiply
                me = finp.tile([P, gb * W], F32, tag="me")
                for bl in range(gb):
                    nc.vector.tensor_scalar_mul(
                        out=me[:, bl * W:(bl + 1) * W], in0=pss[bl][:],
                        scalar1=recb[:, g * gb + bl:g * gb + bl + 1])
                m3 = me[:].rearrange("p (c w) -> p c w", w=W)
                sqm = finp.tile([P, gb * f], F32, tag="sqm")
                q3 = sqm[:].rearrange("p (c e) -> p c e", e=f)
                nc.scalar.square(out=q3[:, :, :], in_=m3[:, :, 0:f])
                va = finp.tile([P, gb * f], F32, tag="va")
                v3 = va[:].rearrange("p (c e) -> p c e", e=f)
                nc.vector.tensor_tensor(
                    out=v3[:, :, :], in0=m3[:, :, f:W], in1=q3[:, :, :],
                    op=AO.subtract)
                nc.vector.tensor_scalar(
                    out=va[:], in0=va[:], scalar1=0.0, scalar2=None,
                    op0=AO.max)
                so = ovp.tile([P, gb * f], F32, tag="so")
                nc.scalar.sqrt(out=so[:], in_=va[:])
                nc.sync.dma_start(
                    out=outd[g * gb * P:(g + 1) * gb * P, :]
                        .rearrange("(c p) e -> p c e", p=P),
                    in_=so[:].rearrange("p (c e) -> p c e", e=f))
    return nc


def _host_prep(x, edge_index):
    import ml_dtypes

    src = np.asarray(edge_index[0], dtype=np.int64)
    tgt = np.asarray(edge_index[1], dtype=np.int64)
    n_edges = src.shape[0]
    counts = np.bincount(tgt, minlength=N_NODES)

    # serpentine deal of count-sorted nodes into NBLK blocks of <=128 slots
    order = np.argsort(-counts, kind="stable")
    ranks = np.arange(N_NODES)
    rounds = ranks // NBLK
    pos = ranks % NBLK
    blk_of_rank = np.where(rounds % 2 == 0, pos, NBLK - 1 - pos)
    blk = np.empty(N_NODES, np.int64)
    slot = np.empty(N_NODES, np.int64)
    blk[order] = blk_of_rank
    slot[order] = rounds
    assert slot.max() < P

    eb = blk[tgt]                      # edge -> block
    es = slot[tgt]                     # edge -> slot in block
    bc = np.bincount(eb, minlength=NBLK)
    starts_b = np.zeros(NBLK, np.int64)
    np.cumsum(bc[:-1], out=starts_b[1:])

    # per block: sort edges by src, split into NRUN equal-count runs
    order_e = np.lexsort((src, eb))
    sb = src[order_e]
    ebo = eb[order_e]
    eso = es[order_e]
    r = np.arange(n_edges) - starts_b[ebo]           # pos within block
    nb_of_e = bc[ebo]                                # block size per edge
    k = (r * NRUN) // nb_of_e                        # run of edge
    w = r - (k * nb_of_e + NRUN - 1) // NRUN         # pos within run
    run_sizes = np.bincount(ebo * NRUN + k, minlength=NBLK * NRUN)
    tq = int(np.ceil(run_sizes.max() / P))
    cap = tq * P
    assert (w >= 0).all() and (w < cap).all()

    # static gather-window bases per run
    bases = []
    for kk in range(NRUN):
        m = k == kk
        lo = int(sb[m].min())
        hi = int(sb[m].max())
        base = min(lo, N_NODES - WIN)
        assert base >= 0 and hi - base < WIN, (kk, lo, hi, base)
        bases.append(base)
    bases = tuple(bases)

    flat = (ebo * NRUN + k) * cap + w
    gidxq = np.zeros((NBLK, NRUN, cap), np.int16)
    gidxq.reshape(-1)[flat] = (sb - np.asarray(bases)[k]).astype(np.int16)

    # rec table: [slot, block] -> 1/count (0 where count<=1 or empty slot)
    rec = np.where(counts >= 2, 1.0 / np.maximum(counts, 1), 0.0)
    recq = np.zeros((NBLK, P), np.float32)
    recq[blk, slot] = rec.astype(np.float32)

    # interleaved rhs table [x | x^2] in bf16
    xf = np.asarray(x, dtype=np.float32)
    xx = np.empty((N_NODES, 2 * N_FEAT), ml_dtypes.bfloat16)
    xx[:, :N_FEAT] = xf
    xx[:, N_FEAT:] = xf * xf
    xx = np.ascontiguousarray(xx)

    ng = NB // GB
    i16c = GB * cap // 16
    C = NB * NRUN * tq

    # per-edge one-hot routing tiles, fp8e4: [P, C, P]; 1 at
    # (partition = w%128, col = (g,k,bl,j), slot)
    core_e = ebo // NB
    bloc = ebo % NB
    g_e = bloc // GB
    bl_e = bloc % GB
    j_e = w // P
    p_e = w % P
    cl_e = g_e * (NRUN * GB * tq) + k * (GB * tq) + bl_e * tq + j_e
    flat_oh = p_e * (C * P) + cl_e * P + eso

    in_maps = []
    for c in range(NCORES):
        gi = gidxq[c * NB:(c + 1) * NB]
        # idx16: per (group, run): stream of GB*cap idxs wrapped %16
        gs = (gi.reshape(ng, GB, NRUN, cap)
              .transpose(0, 2, 1, 3)             # [ng, NRUN, GB, cap]
              .reshape(ng * NRUN, GB * cap))     # per-gather streams
        idx16 = np.ascontiguousarray(
            np.tile(gs.reshape(ng * NRUN, i16c, 16).transpose(0, 2, 1)
                    .reshape(ng * NRUN * 16, i16c)
                    .reshape(ng * NRUN, 16, i16c)
                    .transpose(1, 0, 2).reshape(16, ng * NRUN * i16c),
                    (8, 1)))
        oh_u8 = np.zeros(P * C * P, np.uint8)
        oh_u8[flat_oh[core_e == c]] = FP8_ONE
        oh = oh_u8.view(ml_dtypes.float8_e4m3).reshape(P, C * P)
        in_maps.append({
            "xx": xx,
            "gidx": idx16,
            "oh": oh,
            "rec": np.ascontiguousarray(recq[c * NB:(c + 1) * NB].T),
        })
    return tq, bases, in_maps, blk, slot


def _run(x, edge_index, trace=False):
    from concourse.bass_utils import run_bass_kernel_spmd

    tq, bases, in_maps, blk, slot = _host_prep(x, edge_index)
    key = ("prog", tq, bases, MM_DT, OH_DT)
    if key not in _CACHE:
        nc_ = _build_program(N_NODES, N_FEAT, NB, tq, GB, bases, MM_DT, OH_DT)
        nc_.finalize()
        _CACHE[key] = nc_
    nc = _CACHE[key]
    res = run_bass_kernel_spmd(
        nc, in_maps, core_ids=list(range(NCORES)), trace=trace)

    outs = [np.asarray(r["out"]) for r in res.results]
    out_full = np.empty((N_NODES, N_FEAT), np.float32)
    cores = blk // NB
    rows = (blk % NB) * P + slot
    for c in range(NCORES):
        m = cores == c
        out_full[m] = outs[c][rows[m]]
    return out_full, res


def kernel(**inputs):
    out, _ = _run(inputs["x"], inputs["edge_index"], trace=False)
    return out


# revision 22
# speedup vs baseline: 1.2736x; 1.1663x over previous
"""GNN message-passing (std aggregator) on 8 TRN2 NeuronCores.

Math per target node: count, S1 = sum x[src], S2 = sum x[src]^2;
mean = S1/count; var = S2/count - mean^2; std = sqrt(max(var,0)),
zeroed where count <= 1 (host folds the mask into rec = 1/count).

Strategy: shard TARGET nodes across cores (no collectives). Host packs nodes
into 128-bin blocks balanced by in-degree (serpentine deal). Per block, edges
are sorted by src and split into NRUN equal-count runs; run k of all blocks is
gathered from a 32768-row window (base_k from the data) so int16 gather
indices suffice with ~0 capacity padding. Host also ships (a) an interleaved
bf16 xx = [x | x^2] table so one dma_gather descriptor (256B) fetches a
ready-made rhs row, and (b) the per-edge one-hot routing tiles pre-encoded in
fp8e4 so no engine has to build them. Device per core, per group of GB blocks:
  - 4x dma_gather (one per run; one SWDGE queue each so the 4 Q7 core-pairs
    generate descriptors in parallel) pulls xx[src] rows into SBUF as the
    matmul rhs,
  - a sequential DMA loads the fp8 one-hot tiles,
  - PE matmul-accumulates [128 slots x 128] = [S1 | S2] in per-block PSUM
    banks (zero-region = 2KB bank, so one open group per bank),
  - finishing: per-block PSUM->SBUF copy fused with the 1/count multiply,
    then batched var/sqrt and one DMA out per group.
"""

import numpy as np

N_NODES = 100000
N_FEAT = 64
N_EDGES = 1600000
P = 128
NCORES = 8
NB = 98                 # blocks per core
NBLK = NCORES * NB      # 784
GB = 7                  # blocks per group; 98 = 14*7
NRUN = 4                # equal-count src-sorted runs per block
WIN = 32768             # gather window rows (int16 idx space)
MM_DT = "bfloat16"      # rhs dtype for matmul
OH_DT = "float8e4"      # one-hot dtype (0/1 exact)
FP8_ONE = 0x38          # float8_e4m3 bit pattern of 1.0

_CACHE = {}


def _build_program(n_nodes, f, nb, tq, gb, bases, mm_dt, oh_dt):
    import concourse.bacc as bacc
    import concourse.mybir as mybir
    import concourse.tile as tile

    F32 = mybir.dt.float32
    I16 = mybir.dt.int16
    MDT = getattr(mybir.dt, mm_dt)
    ODT = getattr(mybir.dt, oh_dt)
    AO = mybir.AluOpType

    t = NRUN * tq              # tiles (columns) per block
    W = 2 * f                  # 128
    C = nb * t                 # total columns per core
    gcols = gb * t             # columns per group
    rcols = gb * tq            # columns per (group, run)
    ng = nb // gb
    nidx = rcols * P           # indices per gather
    i16c = nidx // 16          # idx16 cols per gather

    nc = bacc.Bacc(num_swdge_queues=4)
    xxd = nc.declare_dram_parameter("xx", [n_nodes, W], MDT, isOutput=False)
    gidxd = nc.declare_dram_parameter(
        "gidx", [P, ng * NRUN * i16c], I16, isOutput=False)
    ohd = nc.declare_dram_parameter("oh", [P, C * P], ODT, isOutput=False)
    recd = nc.declare_dram_parameter("rec", [P, nb], F32, isOutput=False)
    outd = nc.declare_dram_parameter("out", [nb * P, f], F32, isOutput=True)

    with tile.TileContext(nc) as tc:
        with (
            tc.tile_pool(name="const", bufs=1) as constp,
            tc.tile_pool(name="io", bufs=2) as iop,
            tc.tile_pool(name="msg", bufs=2) as msgp,
            tc.tile_pool(name="fin", bufs=2) as finp,
            tc.tile_pool(name="ov", bufs=2) as ovp,
            tc.tile_pool(name="ps", bufs=8, space="PSUM") as psump,
        ):
            # prefetch the whole int16 index table before anything else so
            # the first gathers start as early as possible
            idxall = constp.tile([P, ng * NRUN * i16c], I16)
            nc.sync.dma_start(out=idxall[:], in_=gidxd[:, :])
            recb = constp.tile([P, nb], F32)
            nc.sync.dma_start(out=recb[:], in_=recd[:, :])

            for g in range(ng):
                sqx = msgp.tile([P, gcols * W], MDT, tag="sqx")
                s3 = sqx[:].rearrange("p (c w) -> p c w", w=W)
                for k in range(NRUN):
                    nc.gpsimd.dma_gather(
                        out_ap=s3[:, k * rcols:(k + 1) * rcols, :],
                        in_ap=xxd[bases[k]:bases[k] + WIN, :],
                        idxs_ap=idxall[:, (g * NRUN + k) * i16c:
                                       (g * NRUN + k + 1) * i16c],
                        num_idxs=nidx,
                        num_idxs_reg=nidx,
                        elem_size=W,
                        single_packet=False,
                        queue_num=k,
                    )
                ohg = msgp.tile([P, gcols * P], ODT, tag="ohg")
                nc.sync.dma_start(
                    out=ohg[:], in_=ohd[:, g * gcols * P:(g + 1) * gcols * P])
                pss = [psump.tile([P, W], F32, tag="ps", name=f"ps_{g}_{bl}")
                       for bl in range(gb)]
                for cl in range(gcols):
                    k = cl // rcols
                    r = cl % rcols
                    bl = r // tq
                    j = r % tq
                    nc.tensor.matmul(
                        out=pss[bl][:],
                        lhsT=ohg[:, cl * P:(cl + 1) * P],
                        rhs=sqx[:, cl * W:(cl + 1) * W],
                        start=(k == 0 and j == 0),
                        stop=(k == NRUN - 1 and j == tq - 1),
                    )
                # finishing for the whole group: [S1|S2] -> std
                # psum -> sbuf copy fused with the 1/count multiply
                me = finp.tile([P, gb * W], F32, tag="me")
                for bl in range(gb):
                    nc.vector.tensor_scalar_mul(
                        out=me[:, bl * W:(bl + 1) * W], in0=pss[bl][:],
                        scalar1=recb[:, g * gb + bl:g * gb + bl + 1])
                m3 = me[:].rearrange("p (c w) -> p c w", w=W)
                sqm = finp.tile([P, gb * f], F32, tag="sqm")
                q3 = sqm[:].rearrange("p (c e) -> p c e", e=f)
                nc.scalar.square(out=q3[:, :, :], in_=m3[:, :, 0:f])
                va = finp.tile([P, gb * f], F32, tag="va")
                v3 = va[:].rearrange("p (c e) -> p c e", e=f)
                nc.vector.tensor_tensor(
                    out=v3[:, :, :], in0=m3[:, :, f:W], in1=q3[:, :, :],
                    op=AO.subtract)
                nc.vector.tensor_scalar(
                    out=va[:], in0=va[:], scalar1=0.0, scalar2=None,
                    op0=AO.max)
                so = ovp.tile([P, gb * f], F32, tag="so")
                nc.scalar.sqrt(out=so[:], in_=va[:])
                nc.sync.dma_start(
                    out=outd[g * gb * P:(g + 1) * gb * P, :]
                        .rearrange("(c p) e -> p c e", p=P),
                    in_=so[:].rearrange("p (c e) -> p c e", e=f))
    return nc


def _host_prep(x, edge_index):
    import ml_dtypes

    src = np.asarray(edge_index[0], dtype=np.int64)
    tgt = np.asarray(edge_index[1], dtype=np.int64)
    n_edges = src.shape[0]
    counts = np.bincount(tgt, minlength=N_NODES)

    # serpentine deal of count-sorted nodes into NBLK blocks of <=128 slots
    order = np.argsort(-counts, kind="stable")
    ranks = np.arange(N_NODES)
    rounds = ranks // NBLK
    pos = ranks % NBLK
    blk_of_rank = np.where(rounds % 2 == 0, pos, NBLK - 1 - pos)
    blk = np.empty(N_NODES, np.int64)
    slot = np.empty(N_NODES, np.int64)
    blk[order] = blk_of_rank
    slot[order] = rounds
    assert slot.max() < P

    eb = blk[tgt]                      # edge -> block
    es = slot[tgt]                     # edge -> slot in block
    bc = np.bincount(eb, minlength=NBLK)
    starts_b = np.zeros(NBLK, np.int64)
    np.cumsum(bc[:-1], out=starts_b[1:])

    # per block: sort edges by src, split into NRUN equal-count runs
    order_e = np.lexsort((src, eb))
    sb = src[order_e]
    ebo = eb[order_e]
    eso = es[order_e]
    r = np.arange(n_edges) - starts_b[ebo]           # pos within block
    nb_of_e = bc[ebo]                                # block size per edge
    k = (r * NRUN) // nb_of_e                        # run of edge
    w = r - (k * nb_of_e + NRUN - 1) // NRUN         # pos within run
    run_sizes = np.bincount(ebo * NRUN + k, minlength=NBLK * NRUN)
    tq = int(np.ceil(run_sizes.max() / P))
    cap = tq * P
    assert (w >= 0).all() and (w < cap).all()

    # static gather-window bases per run
    bases = []
    for kk in range(NRUN):
        m = k == kk
        lo = int(sb[m].min())
        hi = int(sb[m].max())
        base = min(lo, N_NODES - WIN)
        assert base >= 0 and hi - base < WIN, (kk, lo, hi, base)
        bases.append(base)
    bases = tuple(bases)

    flat = (ebo * NRUN + k) * cap + w
    gidxq = np.zeros((NBLK, NRUN, cap), np.int16)
    gidxq.reshape(-1)[flat] = (sb - np.asarray(bases)[k]).astype(np.int16)

    # rec table: [slot, block] -> 1/count (0 where count<=1 or empty slot)
    rec = np.where(counts >= 2, 1.0 / np.maximum(counts, 1), 0.0)
    recq = np.zeros((NBLK, P), np.float32)
    recq[blk, slot] = rec.astype(np.float32)

    # interleaved rhs table [x | x^2] in bf16
    xf = np.asarray(x, dtype=np.float32)
    xx = np.empty((N_NODES, 2 * N_FEAT), ml_dtypes.bfloat16)
    xx[:, :N_FEAT] = xf
    xx[:, N_FEAT:] = xf * xf
    xx = np.ascontiguousarray(xx)

    ng = NB // GB
    i16c = GB * cap // 16
    C = NB * NRUN * tq

    # per-edge one-hot routing tiles, fp8e4: [P, C, P]; 1 at
    # (partition = w%128, col = (g,k,bl,j), slot)
    core_e = ebo // NB
    bloc = ebo % NB
    g_e = bloc // GB
    bl_e = bloc % GB
    j_e = w // P
    p_e = w % P
    cl_e = g_e * (NRUN * GB * tq) + k * (GB * tq) + bl_e * tq + j_e
    flat_oh = p_e * (C * P) + cl_e * P + eso

    in_maps = []
    for c in range(NCORES):
        gi = gidxq[c * NB:(c + 1) * NB]
        # idx16: per (group, run): stream of GB*cap idxs wrapped %16
        gs = (gi.reshape(ng, GB, NRUN, cap)
              .transpose(0, 2, 1, 3)             # [ng, NRUN, GB, cap]
              .reshape(ng * NRUN, GB * cap))     # per-gather streams
        idx16 = np.ascontiguousarray(
            np.tile(gs.reshape(ng * NRUN, i16c, 16).transpose(0, 2, 1)
                    .reshape(ng * NRUN * 16, i16c)
                    .reshape(ng * NRUN, 16, i16c)
                    .transpose(1, 0, 2).reshape(16, ng * NRUN * i16c),
                    (8, 1)))
        oh_u8 = np.zeros(P * C * P, np.uint8)
        oh_u8[flat_oh[core_e == c]] = FP8_ONE
        oh = oh_u8.view(ml_dtypes.float8_e4m3).reshape(P, C * P)
        in_maps.append({
            "xx": xx,
            "gidx": idx16,
            "oh": oh,
            "rec": np.ascontiguousarray(recq[c * NB:(c + 1) * NB].T),
        })
    return tq, bases, in_maps, blk, slot


def _run(x, edge_index, trace=False):
    from concourse.bass_utils import run_bass_kernel_spmd

    tq, bases, in_maps, blk, slot = _host_prep(x, edge_index)
    key = ("prog", tq, bases, MM_DT, OH_DT)
    if key not in _CACHE:
        nc_ = _build_program(N_NODES, N_FEAT, NB, tq, GB, bases, MM_DT, OH_DT)
        nc_.finalize()
        _CACHE[key] = nc_
    nc = _CACHE[key]
    res = run_bass_kernel_spmd(
        nc, in_maps, core_ids=list(range(NCORES)), trace=trace)

    outs = [np.asarray(r["out"]) for r in res.results]
    out_full = np.empty((N_NODES, N_FEAT), np.float32)
    cores = blk // NB
    rows = (blk % NB) * P + slot
    for c in range(NCORES):
        m = cores == c
        out_full[m] = outs[c][rows[m]]
    return out_full, res


def kernel(**inputs):
    out, _ = _run(inputs["x"], inputs["edge_index"], trace=False)
    return out


# revision 23
# speedup vs baseline: 1.2969x; 1.0183x over previous
"""GNN message-passing (std aggregator) on 8 TRN2 NeuronCores.

Math per target node: count, S1 = sum x[src], S2 = sum x[src]^2;
mean = S1/count; var = S2/count - mean^2; std = sqrt(max(var,0)),
zeroed where count <= 1 (host folds the mask into rec = 1/count).

Strategy: shard TARGET nodes across cores (no collectives). Host packs nodes
into 128-bin blocks balanced by in-degree (serpentine deal). Per block, edges
are sorted by src and split into NRUN equal-count runs; run k of all blocks is
gathered from a 32768-row window (base_k from the data) so int16 gather
indices suffice with ~0 capacity padding. Host also ships (a) an interleaved
bf16 xx = [x | x^2] table so one dma_gather descriptor (256B) fetches a
ready-made rhs row, and (b) the per-edge one-hot routing tiles pre-encoded in
fp8e4 so no engine has to build them. Device per core, per group of GB blocks:
  - 4x dma_gather (one per run; one SWDGE queue each so the 4 Q7 core-pairs
    generate descriptors in parallel) pulls xx[src] rows into SBUF as the
    matmul rhs,
  - a sequential DMA loads the fp8 one-hot tiles,
  - PE matmul-accumulates [128 slots x 128] = [S1 | S2] in per-block PSUM
    banks (zero-region = 2KB bank, so one open group per bank),
  - finishing: per-block PSUM->SBUF copy fused with the 1/count multiply,
    then batched var/sqrt and one DMA out per group.
"""

import numpy as np

N_NODES = 100000
N_FEAT = 64
N_EDGES = 1600000
P = 128
NCORES = 8
NB = 98                 # blocks per core
NBLK = NCORES * NB      # 784
GB = 7                  # blocks per group; 98 = 14*7
NRUN = 4                # equal-count src-sorted runs per block
WIN = 32768             # gather window rows (int16 idx space)
MM_DT = "bfloat16"      # rhs dtype for matmul
OH_DT = "float8e4"      # one-hot dtype (0/1 exact)
FP8_ONE = 0x38          # float8_e4m3 bit pattern of 1.0

_CACHE = {}


def _build_program(n_nodes, f, nb, tq, gb, bases, mm_dt, oh_dt):
    import concourse.bacc as bacc
    import concourse.mybir as mybir
    import concourse.tile as tile

    F32 = mybir.dt.float32
    I16 = mybir.dt.int16
    MDT = getattr(mybir.dt, mm_dt)
    ODT = getattr(mybir.dt, oh_dt)
    AO = mybir.AluOpType

    t = NRUN * tq              # tiles (columns) per block
    W = 2 * f                  # 128
    C = nb * t                 # total columns per core
    gcols = gb * t             # columns per group
    rcols = gb * tq            # columns per (group, run)
    ng = nb // gb
    nidx = rcols * P           # indices per gather
    i16c = nidx // 16          # idx16 cols per gather

    nc = bacc.Bacc(num_swdge_queues=4)
    xxd = nc.declare_dram_parameter("xx", [n_nodes, W], MDT, isOutput=False)
    gidxd = nc.declare_dram_parameter(
        "gidx", [P, ng * NRUN * i16c], I16, isOutput=False)
    ohd = nc.declare_dram_parameter("oh", [P, C * P], ODT, isOutput=False)
    recd = nc.declare_dram_parameter("rec", [P, nb], F32, isOutput=False)
    outd = nc.declare_dram_parameter("out", [nb * P, f], F32, isOutput=True)

    with tile.TileContext(nc) as tc:
        with (
            tc.tile_pool(name="const", bufs=1) as constp,
            tc.tile_pool(name="io", bufs=2) as iop,
            tc.tile_pool(name="msg", bufs=2) as msgp,
            tc.tile_pool(name="fin", bufs=2) as finp,
            tc.tile_pool(name="ov", bufs=2) as ovp,
            tc.tile_pool(name="ps", bufs=8, space="PSUM") as psump,
        ):
            # prefetch the int16 index table before anything else so the
            # first gathers start as early as possible; group 0's slice is a
            # separate tile + DMA so its gathers only wait on a tiny load
            gw = NRUN * i16c
            idx0 = constp.tile([P, gw], I16)
            nc.sync.dma_start(out=idx0[:], in_=gidxd[:, :gw])
            idxrest = constp.tile([P, (ng - 1) * gw], I16)
            nc.sync.dma_start(out=idxrest[:], in_=gidxd[:, gw:])
            recb = constp.tile([P, nb], F32)
            nc.sync.dma_start(out=recb[:], in_=recd[:, :])

            for g in range(ng):
                sqx = msgp.tile([P, gcols * W], MDT, tag="sqx")
                s3 = sqx[:].rearrange("p (c w) -> p c w", w=W)
                for k in range(NRUN):
                    if g == 0:
                        iap = idx0[:, k * i16c:(k + 1) * i16c]
                    else:
                        iap = idxrest[:, ((g - 1) * NRUN + k) * i16c:
                                      ((g - 1) * NRUN + k + 1) * i16c]
                    nc.gpsimd.dma_gather(
                        out_ap=s3[:, k * rcols:(k + 1) * rcols, :],
                        in_ap=xxd[bases[k]:bases[k] + WIN, :],
                        idxs_ap=iap,
                        num_idxs=nidx,
                        num_idxs_reg=nidx,
                        elem_size=W,
                        single_packet=False,
                        queue_num=k,
                    )
                ohg = msgp.tile([P, gcols * P], ODT, tag="ohg")
                nc.sync.dma_start(
                    out=ohg[:], in_=ohd[:, g * gcols * P:(g + 1) * gcols * P])
                pss = [psump.tile([P, W], F32, tag="ps", name=f"ps_{g}_{bl}")
                       for bl in range(gb)]
                for cl in range(gcols):
                    k = cl // rcols
                    r = cl % rcols
                    bl = r // tq
                    j = r % tq
                    nc.tensor.matmul(
                        out=pss[bl][:],
                        lhsT=ohg[:, cl * P:(cl + 1) * P],
                        rhs=sqx[:, cl * W:(cl + 1) * W],
                        start=(k == 0 and j == 0),
                        stop=(k == NRUN - 1 and j == tq - 1),
                    )
                # finishing for the whole group: [S1|S2] -> std
                # psum -> sbuf copy fused with the 1/count multiply
                me = finp.tile([P, gb * W], F32, tag="me")
                for bl in range(gb):
                    nc.vector.tensor_scalar_mul(
                        out=me[:, bl * W:(bl + 1) * W], in0=pss[bl][:],
                        scalar1=recb[:, g * gb + bl:g * gb + bl + 1])
                m3 = me[:].rearrange("p (c w) -> p c w", w=W)
                sqm = finp.tile([P, gb * f], F32, tag="sqm")
                q3 = sqm[:].rearrange("p (c e) -> p c e", e=f)
                nc.scalar.square(out=q3[:, :, :], in_=m3[:, :, 0:f])
                va = finp.tile([P, gb * f], F32, tag="va")
                v3 = va[:].rearrange("p (c e) -> p c e", e=f)
                nc.vector.tensor_tensor(
                    out=v3[:, :, :], in0=m3[:, :, f:W], in1=q3[:, :, :],
                    op=AO.subtract)
                nc.vector.tensor_scalar(
                    out=va[:], in0=va[:], scalar1=0.0, scalar2=None,
                    op0=AO.max)
                so = ovp.tile([P, gb * f], F32, tag="so")
                nc.scalar.sqrt(out=so[:], in_=va[:])
                nc.sync.dma_start(
                    out=outd[g * gb * P:(g + 1) * gb * P, :]
                        .rearrange("(c p) e -> p c e", p=P),
                    in_=so[:].rearrange("p (c e) -> p c e", e=f))
    return nc


def _host_prep(x, edge_index):
    import ml_dtypes

    src = np.asarray(edge_index[0], dtype=np.int64)
    tgt = np.asarray(edge_index[1], dtype=np.int64)
    n_edges = src.shape[0]
    counts = np.bincount(tgt, minlength=N_NODES)

    # serpentine deal of count-sorted nodes into NBLK blocks of <=128 slots
    order = np.argsort(-counts, kind="stable")
    ranks = np.arange(N_NODES)
    rounds = ranks // NBLK
    pos = ranks % NBLK
    blk_of_rank = np.where(rounds % 2 == 0, pos, NBLK - 1 - pos)
    blk = np.empty(N_NODES, np.int64)
    slot = np.empty(N_NODES, np.int64)
    blk[order] = blk_of_rank
    slot[order] = rounds
    assert slot.max() < P

    eb = blk[tgt]                      # edge -> block
    es = slot[tgt]                     # edge -> slot in block
    bc = np.bincount(eb, minlength=NBLK)
    starts_b = np.zeros(NBLK, np.int64)
    np.cumsum(bc[:-1], out=starts_b[1:])

    # per block: sort edges by src, split into NRUN equal-count runs
    order_e = np.lexsort((src, eb))
    sb = src[order_e]
    ebo = eb[order_e]
    eso = es[order_e]
    r = np.arange(n_edges) - starts_b[ebo]           # pos within block
    nb_of_e = bc[ebo]                                # block size per edge
    k = (r * NRUN) // nb_of_e                        # run of edge
    w = r - (k * nb_of_e + NRUN - 1) // NRUN         # pos within run
    run_sizes = np.bincount(ebo * NRUN + k, minlength=NBLK * NRUN)
    tq = int(np.ceil(run_sizes.max() / P))
    cap = tq * P
    assert (w >= 0).all() and (w < cap).all()

    # static gather-window bases per run
    bases = []
    for kk in range(NRUN):
        m = k == kk
        lo = int(sb[m].min())
        hi = int(sb[m].max())
        base = min(lo, N_NODES - WIN)
        assert base >= 0 and hi - base < WIN, (kk, lo, hi, base)
        bases.append(base)
    bases = tuple(bases)

    flat = (ebo * NRUN + k) * cap + w
    gidxq = np.zeros((NBLK, NRUN, cap), np.int16)
    gidxq.reshape(-1)[flat] = (sb - np.asarray(bases)[k]).astype(np.int16)

    # rec table: [slot, block] -> 1/count (0 where count<=1 or empty slot)
    rec = np.where(counts >= 2, 1.0 / np.maximum(counts, 1), 0.0)
    recq = np.zeros((NBLK, P), np.float32)
    recq[blk, slot] = rec.astype(np.float32)

    # interleaved rhs table [x | x^2] in bf16
    xf = np.asarray(x, dtype=np.float32)
    xx = np.empty((N_NODES, 2 * N_FEAT), ml_dtypes.bfloat16)
    xx[:, :N_FEAT] = xf
    xx[:, N_FEAT:] = xf * xf
    xx = np.ascontiguousarray(xx)

    ng = NB // GB
    i16c = GB * cap // 16
    C = NB * NRUN * tq

    # per-edge one-hot routing tiles, fp8e4: [P, C, P]; 1 at
    # (partition = w%128, col = (g,k,bl,j), slot)
    core_e = ebo // NB
    bloc = ebo % NB
    g_e = bloc // GB
    bl_e = bloc % GB
    j_e = w // P
    p_e = w % P
    cl_e = g_e * (NRUN * GB * tq) + k * (GB * tq) + bl_e * tq + j_e
    flat_oh = p_e * (C * P) + cl_e * P + eso

    in_maps = []
    for c in range(NCORES):
        gi = gidxq[c * NB:(c + 1) * NB]
        # idx16: per (group, run): stream of GB*cap idxs wrapped %16
        gs = (gi.reshape(ng, GB, NRUN, cap)
              .transpose(0, 2, 1, 3)             # [ng, NRUN, GB, cap]
              .reshape(ng * NRUN, GB * cap))     # per-gather streams
        idx16 = np.ascontiguousarray(
            np.tile(gs.reshape(ng * NRUN, i16c, 16).transpose(0, 2, 1)
                    .reshape(ng * NRUN * 16, i16c)
                    .reshape(ng * NRUN, 16, i16c)
                    .transpose(1, 0, 2).reshape(16, ng * NRUN * i16c),
                    (8, 1)))
        oh_u8 = np.zeros(P * C * P, np.uint8)
        oh_u8[flat_oh[core_e == c]] = FP8_ONE
        oh = oh_u8.view(ml_dtypes.float8_e4m3).reshape(P, C * P)
        in_maps.append({
            "xx": xx,
            "gidx": idx16,
            "oh": oh,
            "rec": np.ascontiguousarray(recq[c * NB:(c + 1) * NB].T),
        })
    return tq, bases, in_maps, blk, slot


def _run(x, edge_index, trace=False):
    from concourse.bass_utils import run_bass_kernel_spmd

    tq, bases, in_maps, blk, slot = _host_prep(x, edge_index)
    key = ("prog", tq, bases, MM_DT, OH_DT)
    if key not in _CACHE:
        nc_ = _build_program(N_NODES, N_FEAT, NB, tq, GB, bases, MM_DT, OH_DT)
        nc_.finalize()
        _CACHE[key] = nc_
    nc = _CACHE[key]
    res = run_bass_kernel_spmd(
        nc, in_maps, core_ids=list(range(NCORES)), trace=trace)

    outs = [np.asarray(r["out"]) for r in res.results]
    out_full = np.empty((N_NODES, N_FEAT), np.float32)
    cores = blk // NB
    rows = (blk % NB) * P + slot
    for c in range(NCORES):
        m = cores == c
        out_full[m] = outs[c][rows[m]]
    return out_full, res


def kernel(**inputs):
    out, _ = _run(inputs["x"], inputs["edge_index"], trace=False)
    return out


# revision 27
# speedup vs baseline: 1.3005x; 1.0028x over previous
"""GNN message-passing (std aggregator) on 8 TRN2 NeuronCores.

Math per target node: count, S1 = sum x[src], S2 = sum x[src]^2;
mean = S1/count; var = S2/count - mean^2; std = sqrt(max(var,0)),
zeroed where count <= 1 (host folds the mask into rec = 1/count).

Strategy: shard TARGET nodes across cores (no collectives). Host packs nodes
into 128-bin blocks balanced by in-degree (serpentine deal). Per block, edges
are sorted by src and split into NRUN equal-count runs; run k of all blocks is
gathered from a 32768-row window (base_k from the data) so int16 gather
indices suffice with ~0 capacity padding. Host also ships (a) an interleaved
bf16 xx = [x | x^2] table so one dma_gather descriptor (256B) fetches a
ready-made rhs row, and (b) the per-edge one-hot routing tiles pre-encoded in
fp8e4 so no engine has to build them. Device per core, per group of GB blocks:
  - 4x dma_gather (one per run; one SWDGE queue each so the 4 Q7 core-pairs
    generate descriptors in parallel) pulls xx[src] rows into SBUF as the
    matmul rhs,
  - a sequential DMA loads the fp8 one-hot tiles,
  - PE matmul-accumulates [128 slots x 128] = [S1 | S2] in per-block PSUM
    banks (zero-region = 2KB bank, so one open group per bank),
  - finishing: per-block PSUM->SBUF copy fused with the 1/count multiply,
    then batched var/sqrt and one DMA out per group.
"""

import numpy as np

N_NODES = 100000
N_FEAT = 64
N_EDGES = 1600000
P = 128
NCORES = 8
NB = 98                 # blocks per core
NBLK = NCORES * NB      # 784
GB = 7                  # blocks per group; 98 = 14*7
NRUN = 4                # equal-count src-sorted runs per block
WIN = 32768             # gather window rows (int16 idx space)
MM_DT = "bfloat16"      # rhs dtype for matmul
OH_DT = "float8e4"      # one-hot dtype (0/1 exact)
FP8_ONE = 0x38          # float8_e4m3 bit pattern of 1.0

_CACHE = {}


def _groups(nb, gb):
    """Blocks per group: full groups of gb, with the tail split in two so
    the final group's matmul/finish drain is short and overlaps the
    second-to-last group's pipeline."""
    n_full = nb // gb - 1
    out = [(i * gb, gb) for i in range(n_full)]
    b0 = n_full * gb
    rem = nb - b0
    a = (rem + 1) // 2
    out.append((b0, a))
    out.append((b0 + a, rem - a))
    return tuple(out)


def _build_program(n_nodes, f, nb, tq, gb, bases, mm_dt, oh_dt):
    import concourse.bacc as bacc
    import concourse.mybir as mybir
    import concourse.tile as tile

    F32 = mybir.dt.float32
    I16 = mybir.dt.int16
    MDT = getattr(mybir.dt, mm_dt)
    ODT = getattr(mybir.dt, oh_dt)
    AO = mybir.AluOpType

    t = NRUN * tq              # tiles (columns) per block
    W = 2 * f                  # 128
    C = nb * t                 # total columns per core
    groups = _groups(nb, gb)
    gbmax = max(n for _, n in groups)
    gcmax = gbmax * t          # max columns per group
    i16_tot = nb * t * P // 16

    nc = bacc.Bacc(num_swdge_queues=4)
    xxd = nc.declare_dram_parameter("xx", [n_nodes, W], MDT, isOutput=False)
    gidxd = nc.declare_dram_parameter("gidx", [P, i16_tot], I16,
                                      isOutput=False)
    ohd = nc.declare_dram_parameter("oh", [P, C * P], ODT, isOutput=False)
    recd = nc.declare_dram_parameter("rec", [P, nb], F32, isOutput=False)
    outd = nc.declare_dram_parameter("out", [nb * P, f], F32, isOutput=True)

    with tile.TileContext(nc) as tc:
        with (
            tc.tile_pool(name="const", bufs=1) as constp,
            tc.tile_pool(name="io", bufs=2) as iop,
            tc.tile_pool(name="msg", bufs=2) as msgp,
            tc.tile_pool(name="fin", bufs=2) as finp,
            tc.tile_pool(name="ov", bufs=2) as ovp,
            tc.tile_pool(name="ps", bufs=8, space="PSUM") as psump,
        ):
            # prefetch the int16 index table before anything else so the
            # first gathers start as early as possible; group 0's slice is a
            # separate tile + DMA so its gathers only wait on a tiny load
            gw = groups[0][1] * t * P // 16
            idx0 = constp.tile([P, gw], I16)
            nc.sync.dma_start(out=idx0[:], in_=gidxd[:, :gw])
            idxrest = constp.tile([P, i16_tot - gw], I16)
            nc.sync.dma_start(out=idxrest[:], in_=gidxd[:, gw:])
            recb = constp.tile([P, nb], F32)
            nc.sync.dma_start(out=recb[:], in_=recd[:, :])

            ioff = 0   # idx16 col offset
            cbase = 0  # one-hot/matmul column offset
            for g, (b0, nbg) in enumerate(groups):
                rcols = nbg * tq       # columns per run
                gcols = nbg * t        # columns in this group
                i16c = rcols * P // 16
                nidx = rcols * P
                sqx = msgp.tile([P, gcmax * W], MDT, tag="sqx")
                s3 = sqx[:].rearrange("p (c w) -> p c w", w=W)
                for k in range(NRUN):
                    if g == 0:
                        iap = idx0[:, k * i16c:(k + 1) * i16c]
                    else:
                        o = ioff - gw + k * i16c
                        iap = idxrest[:, o:o + i16c]
                    nc.gpsimd.dma_gather(
                        out_ap=s3[:, k * rcols:(k + 1) * rcols, :],
                        in_ap=xxd[bases[k]:bases[k] + WIN, :],
                        idxs_ap=iap,
                        num_idxs=nidx,
                        num_idxs_reg=nidx,
                        elem_size=W,
                        single_packet=False,
                        queue_num=k,
                    )
                ohg = msgp.tile([P, gcmax * P], ODT, tag="ohg")
                nc.sync.dma_start(
                    out=ohg[:, :gcols * P],
                    in_=ohd[:, cbase * P:(cbase + gcols) * P])
                pss = [psump.tile([P, W], F32, tag="ps", name=f"ps_{g}_{bl}")
                       for bl in range(nbg)]
                for cl in range(gcols):
                    k = cl // rcols
                    r = cl % rcols
                    bl = r // tq
                    j = r % tq
                    nc.tensor.matmul(
                        out=pss[bl][:],
                        lhsT=ohg[:, cl * P:(cl + 1) * P],
                        rhs=sqx[:, cl * W:(cl + 1) * W],
                        start=(k == 0 and j == 0),
                        stop=(k == NRUN - 1 and j == tq - 1),
                    )
                # finishing for the whole group: [S1|S2] -> std
                # psum -> sbuf copy fused with the 1/count multiply
                me = finp.tile([P, gbmax * W], F32, tag="me")
                for bl in range(nbg):
                    nc.vector.tensor_scalar_mul(
                        out=me[:, bl * W:(bl + 1) * W], in0=pss[bl][:],
                        scalar1=recb[:, b0 + bl:b0 + bl + 1])
                m3 = me[:].rearrange("p (c w) -> p c w", w=W)
                sqm = finp.tile([P, gbmax * f], F32, tag="sqm")
                q3 = sqm[:].rearrange("p (c e) -> p c e", e=f)
                nc.scalar.square(out=q3[:, :nbg, :], in_=m3[:, :nbg, 0:f])
                va = finp.tile([P, gbmax * f], F32, tag="va")
                v3 = va[:].rearrange("p (c e) -> p c e", e=f)
                nc.vector.tensor_tensor(
                    out=v3[:, :nbg, :], in0=m3[:, :nbg, f:W],
                    in1=q3[:, :nbg, :], op=AO.subtract)
                nc.vector.tensor_scalar(
                    out=va[:, :nbg * f], in0=va[:, :nbg * f], scalar1=0.0,
                    scalar2=None, op0=AO.max)
                so = ovp.tile([P, gbmax * f], F32, tag="so")
                nc.scalar.sqrt(out=so[:, :nbg * f], in_=va[:, :nbg * f])
                nc.sync.dma_start(
                    out=outd[b0 * P:(b0 + nbg) * P, :]
                        .rearrange("(c p) e -> p c e", p=P),
                    in_=so[:, :nbg * f].rearrange("p (c e) -> p c e", e=f))
                ioff += NRUN * i16c
                cbase += gcols
    return nc


def _host_prep(x, edge_index):
    import ml_dtypes

    src = np.asarray(edge_index[0], dtype=np.int64)
    tgt = np.asarray(edge_index[1], dtype=np.int64)
    n_edges = src.shape[0]
    counts = np.bincount(tgt, minlength=N_NODES)

    # serpentine deal of count-sorted nodes into NBLK blocks of <=128 slots
    order = np.argsort(-counts, kind="stable")
    ranks = np.arange(N_NODES)
    rounds = ranks // NBLK
    pos = ranks % NBLK
    blk_of_rank = np.where(rounds % 2 == 0, pos, NBLK - 1 - pos)
    blk = np.empty(N_NODES, np.int64)
    slot = np.empty(N_NODES, np.int64)
    blk[order] = blk_of_rank
    slot[order] = rounds
    assert slot.max() < P

    eb = blk[tgt]                      # edge -> block
    es = slot[tgt]                     # edge -> slot in block
    bc = np.bincount(eb, minlength=NBLK)
    starts_b = np.zeros(NBLK, np.int64)
    np.cumsum(bc[:-1], out=starts_b[1:])

    # per block: sort edges by src, split into NRUN equal-count runs
    order_e = np.lexsort((src, eb))
    sb = src[order_e]
    ebo = eb[order_e]
    eso = es[order_e]
    r = np.arange(n_edges) - starts_b[ebo]           # pos within block
    nb_of_e = bc[ebo]                                # block size per edge
    k = (r * NRUN) // nb_of_e                        # run of edge
    w = r - (k * nb_of_e + NRUN - 1) // NRUN         # pos within run
    run_sizes = np.bincount(ebo * NRUN + k, minlength=NBLK * NRUN)
    tq = int(np.ceil(run_sizes.max() / P))
    cap = tq * P
    assert (w >= 0).all() and (w < cap).all()

    # static gather-window bases per run
    bases = []
    for kk in range(NRUN):
        m = k == kk
        lo = int(sb[m].min())
        hi = int(sb[m].max())
        base = min(lo, N_NODES - WIN)
        assert base >= 0 and hi - base < WIN, (kk, lo, hi, base)
        bases.append(base)
    bases = tuple(bases)

    flat = (ebo * NRUN + k) * cap + w
    gidxq = np.zeros((NBLK, NRUN, cap), np.int16)
    gidxq.reshape(-1)[flat] = (sb - np.asarray(bases)[k]).astype(np.int16)

    # rec table: [slot, block] -> 1/count (0 where count<=1 or empty slot)
    rec = np.where(counts >= 2, 1.0 / np.maximum(counts, 1), 0.0)
    recq = np.zeros((NBLK, P), np.float32)
    recq[blk, slot] = rec.astype(np.float32)

    # interleaved rhs table [x | x^2] in bf16
    xf = np.asarray(x, dtype=np.float32)
    xx = np.empty((N_NODES, 2 * N_FEAT), ml_dtypes.bfloat16)
    xx[:, :N_FEAT] = xf
    xx[:, N_FEAT:] = xf * xf
    xx = np.ascontiguousarray(xx)

    groups = _groups(NB, GB)
    C = NB * NRUN * tq

    # per-block (within core) lookup tables for the variable group layout
    bl_of = np.empty(NB, np.int64)
    nbg_of = np.empty(NB, np.int64)
    cbase_of = np.empty(NB, np.int64)
    cb = 0
    for (b0, nbg) in groups:
        bl_of[b0:b0 + nbg] = np.arange(nbg)
        nbg_of[b0:b0 + nbg] = nbg
        cbase_of[b0:b0 + nbg] = cb
        cb += nbg * NRUN * tq
    assert cb == C

    # per-edge one-hot routing tiles, fp8e4: [P, C, P]; 1 at
    # (partition = w%128, col = (group, k, bl, j), slot)
    core_e = ebo // NB
    bloc = ebo % NB
    j_e = w // P
    p_e = w % P
    cl_e = (cbase_of[bloc] + k * (nbg_of[bloc] * tq)
            + bl_of[bloc] * tq + j_e)
    flat_oh = p_e * (C * P) + cl_e * P + eso

    in_maps = []
    for c in range(NCORES):
        gi = gidxq[c * NB:(c + 1) * NB]
        # idx16: per (group, run): stream of nbg*cap idxs wrapped %16
        parts = []
        for (b0, nbg) in groups:
            for kk in range(NRUN):
                stream = gi[b0:b0 + nbg, kk, :].reshape(-1)
                parts.append(stream.reshape(-1, 16).T)   # [16, i16c_gk]
        idx16 = np.ascontiguousarray(
            np.tile(np.hstack(parts), (8, 1)))
        oh_u8 = np.zeros(P * C * P, np.uint8)
        oh_u8[flat_oh[core_e == c]] = FP8_ONE
        oh = oh_u8.view(ml_dtypes.float8_e4m3).reshape(P, C * P)
        in_maps.append({
            "xx": xx,
            "gidx": idx16,
            "oh": oh,
            "rec": np.ascontiguousarray(recq[c * NB:(c + 1) * NB].T),
        })
    return tq, bases, in_maps, blk, slot


def _run(x, edge_index, trace=False):
    from concourse.bass_utils import run_bass_kernel_spmd

    tq, bases, in_maps, blk, slot = _host_prep(x, edge_index)
    key = ("prog", tq, bases, MM_DT, OH_DT)
    if key not in _CACHE:
        nc_ = _build_program(N_NODES, N_FEAT, NB, tq, GB, bases, MM_DT, OH_DT)
        nc_.finalize()
        _CACHE[key] = nc_
    nc = _CACHE[key]
    res = run_bass_kernel_spmd(
        nc, in_maps, core_ids=list(range(NCORES)), trace=trace)

    outs = [np.asarray(r["out"]) for r in res.results]
    out_full = np.empty((N_NODES, N_FEAT), np.float32)
    cores = blk // NB
    rows = (blk % NB) * P + slot
    for c in range(NCORES):
        m = cores == c
        out_full[m] = outs[c][rows[m]]
    return out_full, res


def kernel(**inputs):
    out, _ = _run(inputs["x"], inputs["edge_index"], trace=False)
    return out
